# revision 5
# baseline (speedup 1.0000x reference)
"""AWQ W4A8 linear (x:[8,32,8192] f32, qweight:[8192,8192] int4-range int32,
w_scales/bias:[8192] f32) -> [8,32,8192] f32 on 8 trn2 NeuronCores.

Column-parallel sharding: qweight/w_scales/bias split along N across the 8
cores; x (quantized per-token on host, exactly as the reference does) and
act_scales are replicated. Each core computes an exact integer GEMM of
x_q [256,8192] @ qw_shard [8192,1024], applies the per-token/per-channel
dequant + bias epilogue, and writes its [256,1024] output slice. Host
concatenates the slices.

Numerics: x_q in [-127,127] ships as bf16, qw in [-8,7] ships as fp8e4 —
both exact — and the PE's mixed bf16 x fp8 matmul accumulates exactly in
fp32 PSUM (all products/sums are integers < 2^24), so the result matches
the reference bit-for-bit while weight HBM traffic drops 4x vs int32.

Schedule: weight DMA groups ramp 1,1,2,4,4... k-chunks so the first matmul
only waits on a 64KB activation tile and a 128KB weight tile; the dequant
epilogue is split across DVE and ACT+GpSimd so the post-GEMM tail stays
short.
"""

import numpy as np

import concourse.bass as bass
import concourse.bacc as bacc
import concourse.mybir as mybir
import concourse.tile as tile
import concourse.bass_utils as bass_utils
from concourse.dt import dt as cdt

N_CORES = 8
P = 128
B, S, K, N = 8, 32, 8192, 8192
TOK = B * S                      # 256 tokens
NL = N // N_CORES                # 1024 output channels per core
KC = K // P                      # 64 contraction chunks of 128
EPS = 1e-8

# weight-stream group sizes (k-chunks per DMA); ramped so the PE starts early
W_GROUPS = [1, 1, 2] + [4] * 15
assert sum(W_GROUPS) == KC

# activation tile split (k-chunks per resident tile)
X_GROUPS = [1, 7] + [8] * 7
assert sum(X_GROUPS) == KC

_cached_nc = None


def _x_tile_of(c):
    """chunk index -> (tile_idx, offset within tile)"""
    acc = 0
    for i, gc in enumerate(X_GROUPS):
        if c < acc + gc:
            return i, c - acc
        acc += gc
    raise ValueError(c)


def _build_nc():
    nc = bacc.Bacc(
        "TRN2",
        target_bir_lowering=False,
        debug=False,
        enable_asserts=False,
        num_devices=N_CORES,
    )
    dt = mybir.dt

    xq_d = nc.dram_tensor("xq", [P, KC, TOK], dt.bfloat16, kind="ExternalInput")
    qw_d = nc.dram_tensor("qw", [KC, P, NL], dt.float8e4, kind="ExternalInput")
    ws_d = nc.dram_tensor("ws", [P, NL], dt.float32, kind="ExternalInput")
    bs_d = nc.dram_tensor("bs", [P, NL], dt.float32, kind="ExternalInput")
    as_d = nc.dram_tensor("asc", [P, 2], dt.float32, kind="ExternalInput")
    out_d = nc.dram_tensor("out", [2, P, NL], dt.float32, kind="ExternalOutput")

    with tile.TileContext(nc) as tc:
        with (
            tc.tile_pool(name="xp", bufs=1) as xp,
            tc.tile_pool(name="wp", bufs=3) as wp,
            tc.tile_pool(name="cst", bufs=1) as cst,
            tc.tile_pool(name="op", bufs=4) as op,
            tc.tile_pool(name="pp", bufs=1, space="PSUM") as pp,
        ):
            # Resident activations. Tile 0 holds just chunk 0 (64KB) so the
            # PE can start almost immediately; issued first on SP's queue.
            xq_t = []
            xq_off = []
            acc = 0
            for i, gc in enumerate(X_GROUPS):
                t = xp.tile([P, gc, TOK], dt.bfloat16, name=f"xq{i}", tag=f"xq{i}")
                xq_t.append(t)
                xq_off.append(acc)
                acc += gc
            nc.sync.dma_start(xq_t[0][:], xq_d.ap()[:, 0:1, :])

            ws_s = cst.tile([P, NL], dt.float32, name="ws_s", tag="ws")
            bs_s = cst.tile([P, NL], dt.float32, name="bs_s", tag="bs")
            as_s = cst.tile([P, 2], dt.float32, name="as_s", tag="asc")
            # everything not on the critical path goes on ACT's DGE queue
            for i in range(1, len(X_GROUPS)):
                o = xq_off[i]
                nc.scalar.dma_start(
                    xq_t[i][:], xq_d.ap()[:, o : o + X_GROUPS[i], :]
                )
            nc.scalar.dma_start(ws_s[:], ws_d.ap())
            nc.scalar.dma_start(bs_s[:], bs_d.ap())
            nc.scalar.dma_start(as_s[:], as_d.ap())

            ps = {}
            for m in range(2):
                for n in range(2):
                    ps[(m, n)] = pp.tile(
                        [P, 512], dt.float32, name=f"ps{m}{n}", tag=f"ps{m}{n}"
                    )

            c0 = 0
            for g, gc in enumerate(W_GROUPS):
                wt = wp.tile([P, 4, NL], dt.float8e4, name="wt", tag="wt")
                nc.sync.dma_start(
                    wt[:, :gc, :],
                    qw_d.ap()[c0 : c0 + gc].rearrange("c p n -> p c n"),
                )
                for j in range(gc):
                    c = c0 + j
                    ti, to = _x_tile_of(c)
                    for m in range(2):
                        lhsT = xq_t[ti][:, to, P * m : P * (m + 1)]
                        for n in range(2):
                            nc.tensor.matmul(
                                ps[(m, n)][:],
                                lhsT,
                                wt[:, j, 512 * n : 512 * (n + 1)],
                                start=(c == 0),
                                stop=(c == KC - 1),
                            )
                c0 += gc

            # Dequant epilogue, split across engines so the tail after the
            # last matmul is short. m=1 tiles: ACT applies the per-token
            # scale while copying PSUM->SBUF, GpSimd does the per-channel
            # scale + bias. m=0 tiles: DVE does everything.
            for n in range(2):
                nsl = slice(512 * n, 512 * (n + 1))
                t1 = op.tile([P, 512], dt.float32, name="t1", tag="t1")
                nc.scalar.mul(t1[:], ps[(1, n)][:], as_s[:, 1:2])
                t2 = op.tile([P, 512], dt.float32, name="t2", tag="t2")
                nc.gpsimd.tensor_mul(t2[:], t1[:], ws_s[:, nsl])
                o1 = op.tile([P, 512], dt.float32, name="o1", tag="o1")
                nc.gpsimd.tensor_add(o1[:], t2[:], bs_s[:, nsl])
                nc.scalar.dma_start(out_d.ap()[1][:, nsl], o1[:])

                t3 = op.tile([P, 512], dt.float32, name="t3", tag="t3")
                nc.vector.scalar_tensor_tensor(
                    t3[:],
                    ps[(0, n)][:],
                    as_s[:, 0:1],
                    ws_s[:, nsl],
                    mybir.AluOpType.mult,
                    mybir.AluOpType.mult,
                )
                o2 = op.tile([P, 512], dt.float32, name="o2", tag="o2")
                nc.vector.tensor_add(o2[:], t3[:], bs_s[:, nsl])
                nc.sync.dma_start(out_d.ap()[0][:, nsl], o2[:])

    nc.compile()
    return nc


def _prep_inputs(x, qweight, w_scales, bias):
    bf16 = cdt.np(mybir.dt.bfloat16)
    fp8 = cdt.np(mybir.dt.float8e4)

    x2 = np.asarray(x, dtype=np.float32).reshape(TOK, K)
    max_abs = np.max(np.abs(x2), axis=-1, keepdims=True)
    act_scales = np.maximum(max_abs / np.float32(127.0), np.float32(EPS)).astype(
        np.float32
    )
    x_q = np.clip(np.round(x2 / act_scales), -127, 127).astype(np.float32)

    # [TOK, K] -> K-major [P, KC, TOK]: xq[p, c, t] = x_q[t, c*128 + p]
    xq = np.ascontiguousarray(
        x_q.T.reshape(KC, P, TOK).transpose(1, 0, 2).astype(bf16)
    )

    # act_scales arranged per m-tile: asc[p, m] = act_scales[m*128 + p]
    asc = np.ascontiguousarray(act_scales.reshape(2, P).T.astype(np.float32))

    # int4-range weights are exactly representable in fp8 e4m3
    qw8 = np.asarray(qweight, dtype=np.int8).astype(fp8)
    w_scales = np.asarray(w_scales, dtype=np.float32)
    bias = np.asarray(bias, dtype=np.float32)

    in_maps = []
    for i in range(N_CORES):
        sl = slice(i * NL, (i + 1) * NL)
        # [K, NL] -> [KC, P, NL]: qw[c, p, n] = shard[c*128 + p, n]
        shard = qw8[:, sl].reshape(KC, P, NL)
        in_maps.append(
            {
                "xq": xq,
                "qw": np.ascontiguousarray(shard),
                "ws": np.ascontiguousarray(
                    np.broadcast_to(w_scales[sl][None, :], (P, NL))
                ),
                "bs": np.ascontiguousarray(
                    np.broadcast_to(bias[sl][None, :], (P, NL))
                ),
                "asc": asc,
            }
        )
    return in_maps


def kernel(x, qweight, w_scales, bias):
    global _cached_nc
    if _cached_nc is None:
        _cached_nc = _build_nc()
    nc = _cached_nc

    in_maps = _prep_inputs(x, qweight, w_scales, bias)
    res = bass_utils.run_bass_kernel_spmd(
        nc, in_maps, core_ids=list(range(N_CORES))
    )

    out = np.empty((TOK, N), dtype=np.float32)
    for i in range(N_CORES):
        out[:, i * NL : (i + 1) * NL] = res.results[i]["out"].reshape(TOK, NL)
    return out.reshape(B, S, N)


# revision 8
# speedup vs baseline: 1.1253x; 1.1253x over previous
"""AWQ W4A8 linear (x:[8,32,8192] f32, qweight:[8192,8192] int4-range int32,
w_scales/bias:[8192] f32) -> [8,32,8192] f32 on 8 trn2 NeuronCores.

Column-parallel sharding: qweight/w_scales/bias split along N across the 8
cores; x (quantized per-token on host, exactly as the reference does) and
act_scales are replicated. Each core computes an exact integer GEMM of
x_q [256,8192] @ qw_shard [8192,1024], applies the per-token/per-channel
dequant + bias epilogue, and writes its [256,1024] output slice. Host
concatenates the slices.

Numerics: x_q in [-127,127] ships as bf16, qw in [-8,7] ships as fp8e4 —
both exact — and the PE's mixed bf16 x fp8 matmul accumulates exactly in
fp32 PSUM (all products/sums are integers < 2^24), so the result matches
the reference bit-for-bit while weight HBM traffic drops 4x vs int32.

Schedule: a few dummy matmuls warm the PE clock (HAM) during the initial
DMA wait; the weight stream uses ramped group sizes with triple buffering;
the last weight group runs tile-by-tile so the four dequant epilogues and
stores overlap the final matmuls.
"""

import numpy as np

import concourse.bass as bass
import concourse.bacc as bacc
import concourse.mybir as mybir
import concourse.tile as tile
import concourse.bass_utils as bass_utils
from concourse.dt import dt as cdt

N_CORES = 8
P = 128
B, S, K, N = 8, 32, 8192, 8192
TOK = B * S                      # 256 tokens
NL = N // N_CORES                # 1024 output channels per core
KC = K // P                      # 64 contraction chunks of 128
EPS = 1e-8

W_GROUPS = [2, 2, 4] + [8] * 7   # weight k-chunks per DMA group
X_GROUPS = [1, 7, 14, 14, 14, 14]  # activation k-chunks per resident tile
N_WARM = 14                      # dummy matmuls to warm the PE clock gate
assert sum(W_GROUPS) == KC and sum(X_GROUPS) == KC

_cached_nc = None


def _x_tile_of(c):
    acc = 0
    for i, gc in enumerate(X_GROUPS):
        if c < acc + gc:
            return i, c - acc
        acc += gc
    raise ValueError(c)


def _build_nc():
    nc = bacc.Bacc(
        "TRN2",
        target_bir_lowering=False,
        debug=False,
        enable_asserts=False,
        num_devices=N_CORES,
    )
    dt = mybir.dt

    xq_d = nc.dram_tensor("xq", [P, KC, TOK], dt.bfloat16, kind="ExternalInput")
    qw_d = nc.dram_tensor("qw", [P, KC, NL], dt.float8e4, kind="ExternalInput")
    ws_d = nc.dram_tensor("ws", [P, NL], dt.float32, kind="ExternalInput")
    bs_d = nc.dram_tensor("bs", [P, NL], dt.float32, kind="ExternalInput")
    as_d = nc.dram_tensor("asc", [P, 2], dt.float32, kind="ExternalInput")
    out_d = nc.dram_tensor("out", [2, P, NL], dt.float32, kind="ExternalOutput")

    with tile.TileContext(nc) as tc:
        with (
            tc.tile_pool(name="xp", bufs=1) as xp,
            tc.tile_pool(name="wp", bufs=3) as wp,
            tc.tile_pool(name="cst", bufs=1) as cst,
            tc.tile_pool(name="op", bufs=4) as op,
            tc.tile_pool(name="pp", bufs=1, space="PSUM") as pp,
        ):
            # critical-path DMAs first on SP's queue: first weight group,
            # then the first activation chunk
            gc0 = W_GROUPS[0]
            wt0 = wp.tile([P, 8, NL], dt.float8e4, name="wt", tag="wt")
            nc.sync.dma_start(wt0[:, :gc0, :], qw_d.ap()[:, 0:gc0, :])

            xq_t = []
            acc = 0
            for i, gc in enumerate(X_GROUPS):
                t = xp.tile([P, gc, TOK], dt.bfloat16, name=f"xq{i}", tag=f"xq{i}")
                xq_t.append((t, acc))
                acc += gc
            nc.sync.dma_start(xq_t[0][0][:], xq_d.ap()[:, 0:1, :])

            # off-critical-path loads on ACT's DGE queue
            for i in range(1, len(X_GROUPS)):
                t, o = xq_t[i]
                nc.scalar.dma_start(t[:], xq_d.ap()[:, o : o + X_GROUPS[i], :])
            ws_s = cst.tile([P, NL], dt.float32, name="ws_s", tag="ws")
            bs_s = cst.tile([P, NL], dt.float32, name="bs_s", tag="bs")
            as_s = cst.tile([P, 2], dt.float32, name="as_s", tag="asc")
            nc.scalar.dma_start(ws_s[:], ws_d.ap())
            nc.scalar.dma_start(bs_s[:], bs_d.ap())
            nc.scalar.dma_start(as_s[:], as_d.ap())

            ps = {}
            for m in range(2):
                for n in range(2):
                    ps[(m, n)] = pp.tile(
                        [P, 512], dt.float32, name=f"ps{m}{n}", tag=f"ps{m}{n}"
                    )

            # PE warmup: dependency-free matmuls on a zeroed scratch tile so
            # the HAM clock gate opens while the first real DMAs are in
            # flight. Results go to a scratch PSUM bank that is never read.
            warm = cst.tile([P, 512], dt.bfloat16, name="warm", tag="warm")
            ps_w = pp.tile([P, 512], dt.float32, name="psw", tag="psw")
            nc.gpsimd.memset(warm[:], 0.0)
            for _ in range(N_WARM):
                nc.tensor.matmul(
                    ps_w[:], warm[:, :P], warm[:], start=True, stop=True
                )

            def mm(c, m, n, wt, j):
                xt, xo = xq_t[_x_tile_of(c)[0]]
                lhsT = xt[:, c - xo, P * m : P * (m + 1)]
                nc.tensor.matmul(
                    ps[(m, n)][:],
                    lhsT,
                    wt[:, j, 512 * n : 512 * (n + 1)],
                    start=(c == 0),
                    stop=(c == KC - 1),
                )

            # main stream: (chunk, m, n) inner order keeps one LDWEIGHTS per
            # pair of matmuls
            c0 = 0
            for g, gc in enumerate(W_GROUPS[:-1]):
                if g == 0:
                    wt = wt0
                else:
                    wt = wp.tile([P, 8, NL], dt.float8e4, name="wt", tag="wt")
                    nc.sync.dma_start(wt[:, :gc, :], qw_d.ap()[:, c0 : c0 + gc, :])
                for j in range(gc):
                    for m in range(2):
                        for n in range(2):
                            mm(c0 + j, m, n, wt, j)
                c0 += gc

            # last group tile-by-tile so each PSUM tile finishes (and its
            # epilogue + store runs) while the next tile's matmuls stream
            gc = W_GROUPS[-1]
            wt = wp.tile([P, 8, NL], dt.float8e4, name="wt", tag="wt")
            nc.sync.dma_start(wt[:, :gc, :], qw_d.ap()[:, c0 : c0 + gc, :])
            store_eng = [nc.sync, nc.scalar]
            for idx, (m, n) in enumerate([(0, 0), (0, 1), (1, 0), (1, 1)]):
                for j in range(gc):
                    mm(c0 + j, m, n, wt, j)
                nsl = slice(512 * n, 512 * (n + 1))
                t1 = op.tile([P, 512], dt.float32, name="t1", tag="t1")
                nc.vector.scalar_tensor_tensor(
                    t1[:],
                    ps[(m, n)][:],
                    as_s[:, m : m + 1],
                    ws_s[:, nsl],
                    mybir.AluOpType.mult,
                    mybir.AluOpType.mult,
                )
                o1 = op.tile([P, 512], dt.float32, name="o1", tag="o1")
                nc.vector.tensor_add(o1[:], t1[:], bs_s[:, nsl])
                store_eng[idx % 2].dma_start(out_d.ap()[m][:, nsl], o1[:])

    nc.compile()
    return nc


def _prep_inputs(x, qweight, w_scales, bias):
    bf16 = cdt.np(mybir.dt.bfloat16)
    fp8 = cdt.np(mybir.dt.float8e4)

    x2 = np.asarray(x, dtype=np.float32).reshape(TOK, K)
    max_abs = np.max(np.abs(x2), axis=-1, keepdims=True)
    act_scales = np.maximum(max_abs / np.float32(127.0), np.float32(EPS)).astype(
        np.float32
    )
    x_q = np.clip(np.round(x2 / act_scales), -127, 127).astype(np.float32)

    # [TOK, K] -> K-major [P, KC, TOK]: xq[p, c, t] = x_q[t, c*128 + p]
    xq = np.ascontiguousarray(
        x_q.T.reshape(KC, P, TOK).transpose(1, 0, 2).astype(bf16)
    )

    # act_scales arranged per m-tile: asc[p, m] = act_scales[m*128 + p]
    asc = np.ascontiguousarray(act_scales.reshape(2, P).T.astype(np.float32))

    # int4-range weights are exactly representable in fp8 e4m3
    qw8 = np.asarray(qweight, dtype=np.int8).astype(fp8)
    w_scales = np.asarray(w_scales, dtype=np.float32)
    bias = np.asarray(bias, dtype=np.float32)

    in_maps = []
    for i in range(N_CORES):
        sl = slice(i * NL, (i + 1) * NL)
        # [K, NL] -> p-major [P, KC, NL]: qw[p, c, n] = shard[c*128 + p, n]
        shard = qw8[:, sl].reshape(KC, P, NL).transpose(1, 0, 2)
        in_maps.append(
            {
                "xq": xq,
                "qw": np.ascontiguousarray(shard),
                "ws": np.ascontiguousarray(
                    np.broadcast_to(w_scales[sl][None, :], (P, NL))
                ),
                "bs": np.ascontiguousarray(
                    np.broadcast_to(bias[sl][None, :], (P, NL))
                ),
                "asc": asc,
            }
        )
    return in_maps


def kernel(x, qweight, w_scales, bias):
    global _cached_nc
    if _cached_nc is None:
        _cached_nc = _build_nc()
    nc = _cached_nc

    in_maps = _prep_inputs(x, qweight, w_scales, bias)
    res = bass_utils.run_bass_kernel_spmd(
        nc, in_maps, core_ids=list(range(N_CORES))
    )

    out = np.empty((TOK, N), dtype=np.float32)
    for i in range(N_CORES):
        out[:, i * NL : (i + 1) * NL] = res.results[i]["out"].reshape(TOK, NL)
    return out.reshape(B, S, N)
